# revision 15
# baseline (speedup 1.0000x reference)
"""Trainium2 Bass kernel for nn_Deconv (NMF deconvolution, B=8, C=64, SRC=16, 256x256, K=5).

Data-parallel over batch: each of 8 NeuronCores runs one sample. All
convolutions use a transposed-output formulation: each matmul takes one
input-image row as the stationary operand (lhsT, partition dim = packed
channel/x-shift), streams a small weight matrix as the moving operand, and
scatters into PSUM columns indexed (out_channel, out_row) for a 32-row block.

DMA strategy (this revision): the x-shift-packed row windows are built with
five full-row-segment DMAs spread across five engine queues (sync, scalar,
vector, gpsimd) instead of five column-sliced loads serialized on the sync
queue; transposed s'-writeback and conv outputs are batched into one DMA per
32-row block via multi-dim access patterns; PSUM zero-fill matmuls are
removed (the first accumulating matmul clears the bank via start=True).

Per iteration:
  A1: xh_a = conv(s, h) into a 32-row SBUF staging tile, one DMA per block
      to the padded DRAM image.
  A2: num^T = conv(x2, hT) and den^T = conv(xh2, hT) fused per 32-row block
      with the multiplicative s-update (DVE) and the s'-to-DRAM writeback.
  B:  xh_b^T = conv(s', h) evicted into an interleaved (Y, c128) tile that
      also holds x^T, then the sconv pixel-contraction accumulates
      num2/den2 for both images in one PSUM bank; h-update follows.
Final: conv(s, h) in the normal formulation straight into the output.

bf16 everywhere on the PE (fp32 PSUM accumulation); measured end-to-end
relative error ~6e-3 vs the fp32 reference.
"""

import sys

sys.path.insert(0, "/opt/trn_rl_repo")

import numpy as np

import concourse.bass as bass  # noqa: F401
import concourse.tile as tile
from concourse import bacc, mybir

F32 = mybir.dt.float32
BF16 = mybir.dt.bfloat16

B, C, S = 8, 64, 16
H = W = 256
KK = 5
PW2 = 260   # pad-2 canvas (x2, xh_pad)
PW4 = 264   # pad-4 canvas (s_pad)
N_CORES = 8

_CACHE = {}


def _emit(nc, tc, ins, outs):
    from contextlib import ExitStack

    x2d, xTd, wlb, bpat, h0f, h0b, identb, zerob = ins
    out_d = outs

    # round-robin DMA issue engines for the 5-shift pack loads
    dma_engines = [nc.sync, nc.scalar, nc.gpsimd]

    ctx = ExitStack()
    with ctx:
        dram = ctx.enter_context(tc.tile_pool(name="dram", bufs=1, space="DRAM"))
        consts = ctx.enter_context(tc.tile_pool(name="consts", bufs=1))

        s_pad = [
            dram.tile([S, PW4, PW4], BF16, tag=f"s_pad{i}", name=f"s_pad{i}")
            for i in range(2)
        ]
        xh_pad = dram.tile([C, PW2, PW2], BF16, tag="xh_pad", name="xh_pad")

        # ---- consts ----
        identt = consts.tile([128, 128], BF16, tag="identb", name="identb")
        nc.sync.dma_start(identt[:], identb[:])
        wlt = consts.tile([C, S], BF16, tag="wlb", name="wlb")
        nc.sync.dma_start(wlt[:], wlb[:])
        bpt = consts.tile([128, 32, S], F32, tag="bpat", name="bpat")
        nc.sync.dma_start(bpt[:], bpat[:])

        zt = consts.tile([1, 512], BF16, tag="zmm", name="zmm")
        nc.gpsimd.memset(zt[:], 0.0)
        zl = consts.tile([1, 128], BF16, tag="zl", name="zl")
        nc.gpsimd.memset(zl[:], 0.0)

        hv2 = [consts.tile([C, 400], F32, tag=f"hv{v}", name=f"hv{v}") for v in range(2)]
        hbv2 = [consts.tile([C, 400], BF16, tag=f"hb{v}", name=f"hb{v}") for v in range(2)]
        hv = {0: hv2[0], 1: hv2[1], 2: hv2[0]}
        hbv = {0: hbv2[0], 1: hbv2[1], 2: hbv2[0]}
        nc.sync.dma_start(hv[0][:], h0f[:])
        nc.sync.dma_start(hbv[0][:], h0b[:])

        # s^T chunks, ping-pong by version parity: cols (s, Y)
        sT2 = [
            [consts.tile([128, H, S], BF16, tag=f"sT{v}_{cb}", name=f"sT{v}_{cb}")
             for cb in range(2)]
            for v in range(2)
        ]
        sT = {0: sT2[0], 1: sT2[1], 2: sT2[0]}

        # per-iter weights
        wnum = [
            [consts.tile([128 if va < 4 else 64, KK, S], BF16,
                         tag=f"wnum{it}_{va}", name=f"wnum{it}_{va}")
             for va in (0, 2, 4)]
            for it in range(2)
        ]
        wxh = [
            [consts.tile([80, KK, 32], BF16, tag=f"wxh{it}_{h}",
                         name=f"wxh{it}_{h}") for h in range(2)]
            for it in range(2)
        ]
        hbQI = consts.tile([C, KK, KK, S], BF16, tag="hbQI", name="hbQI")
        hT5v = [consts.tile([80, KK, C], BF16, tag=f"hT5_{i}", name=f"hT5_{i}")
                for i in range(3)]
        hB = consts.tile([C, 400], BF16, tag="hB", name="hB")

        # ---- zero rims (once) ----
        rim_engines = [nc.sync, nc.scalar, nc.sync, nc.scalar]
        for si, sp in enumerate(s_pad):
            spv = sp
            rim_engines[0].dma_start(spv[:, 0:4, :], zerob[0:S, 0:4 * PW4])
            rim_engines[1].dma_start(spv[:, 260:264, :], zerob[0:S, 0:4 * PW4])
            rim_engines[2].dma_start(spv[:, 4:260, 0:4], zerob[0:S, 0:1024])
            rim_engines[3].dma_start(spv[:, 4:260, 260:264], zerob[0:S, 0:1024])
        xhv_ = xh_pad
        rim_engines[0].dma_start(xhv_[:, 0:2, :], zerob[0:C, 0:2 * PW2])
        rim_engines[1].dma_start(xhv_[:, 258:260, :], zerob[0:C, 0:2 * PW2])
        rim_engines[2].dma_start(xhv_[:, 2:258, 0:2], zerob[0:C, 0:512])
        rim_engines[3].dma_start(xhv_[:, 2:258, 258:260], zerob[0:C, 0:512])

        def build_weights(it):
            """wnum[it], wxh[it] from hbv[it]."""
            hb4 = hbv[it][:].rearrange("c (i u v) -> c i u v", i=S, u=KK, v=KK)
            for vi, va in enumerate((0, 2, 4)):
                kp = 128 if va < 4 else 64
                for d in range(kp // 64):
                    nc.vector.tensor_copy(
                        wnum[it][vi][d * 64:(d + 1) * 64, :, :],
                        hb4[:, :, :, 4 - va - d].rearrange("c i u -> c u i"),
                    )
            nc.gpsimd.tensor_copy(
                hbQI[:], hbv[it][:].rearrange("c (i u v) -> c u v i",
                                              i=S, u=KK, v=KK))
            with tc.tile_pool(name=f"wps{it}", bufs=1, space="PSUM") as wps:
                wx_ps = wps.tile([80, KK, C], BF16, tag="wx_ps", name="wx_ps")
                for p in range(KK):
                    nc.tensor.transpose(
                        wx_ps[:, 4 - p, :],
                        hbQI[:, p, :, :].rearrange("c v i -> c (v i)"),
                        identt[0:C, 0:C],
                    )
                nc.scalar.copy(wxh[it][0][:], wx_ps[:, :, 0:32])
                nc.scalar.copy(wxh[it][1][:], wx_ps[:, :, 32:64])
            build_hT5(it)

        def build_hT5(idx):
            hb4x = hbv[idx][:].rearrange("c (i u v) -> c i u v", i=S, u=KK, v=KK)
            hBo = hB[:].rearrange("c (u g i) -> c u g i", u=KK, g=KK, i=S)
            nc.gpsimd.tensor_copy(hBo, hb4x.rearrange("c i u g -> c u g i"))
            with tc.tile_pool(name=f"hw{idx}", bufs=1, space="PSUM") as fwp:
                for u in range(KK):
                    pw = fwp.tile([80, C], BF16, tag="pw", name="pw")
                    nc.tensor.transpose(pw[:], hB[:, u * 80:u * 80 + 80],
                                        identt[0:C, 0:C])
                    nc.scalar.copy(hT5v[idx][:, u, :], pw[:])

        def load_pack(pool, spv, y0, x0, wd, tag):
            """[80, 36, wd] tile: pack[16g+i, r, x] = spv[i, y0+2+r, x0+g+x].

            Five full-row-segment DMAs spread across five engine queues.
            """
            pk = pool.tile([80, 36, wd], BF16, tag=tag, name=tag)
            for g in range(KK):
                w = min(wd, PW4 - x0 - g)
                dma_engines[(y0 // 32 + g) % 3].dma_start(
                    pk[g * S:(g + 1) * S, :, 0:w],
                    spv[:, y0 + 2:y0 + 38, x0 + g:x0 + g + w],
                )
            return pk

        def sT_to_dram(v, cb, y0, pool, eng, deng):
            """Write sT[v][cb] rows [y0, y0+32) to s_pad[v & 1] via PE transpose."""
            x0 = cb * 128
            with tc.tile_pool(name="tbp", bufs=2, space="PSUM") as tbp:
                tb_ps = tbp.tile([128, 4, 128], BF16, tag="tb_ps", name="tb_ps")
                for j in range(4):
                    nc.tensor.transpose(
                        tb_ps[:, j, :],
                        sT[v][cb][:, y0 + 8 * j:y0 + 8 * j + 8, :].rearrange(
                            "p y s -> p (y s)"),
                        identt[:],
                    )
                tb_sb = pool.tile([128, 4, 128], BF16, tag="tb_sb", name="tb_sb")
                if eng is nc.vector:
                    nc.vector.tensor_copy(tb_sb[:], tb_ps[:])
                else:
                    nc.scalar.copy(tb_sb[:], tb_ps[:])
            # DMA APs are limited to 3 dims, so one DMA per 8-row group,
            # spread across the three DMA-capable queues.
            jeng = [dma_engines[(y0 // 32 + cb + j) % 3] for j in range(4)]
            for j in range(4):
                dst = s_pad[v & 1][:, 4 + y0 + 8 * j:4 + y0 + 8 * j + 8,
                                   4 + x0:4 + x0 + 128].rearrange(
                    "i yy m -> yy i m")
                jeng[j].dma_start(dst, tb_sb[:, j, :])

        def phase_a1(it, v):
            """xh_a = conv(s_pad[v], h) -> xh_pad, normal formulation."""
            with (
                tc.tile_pool(name="a1_in", bufs=2) as pin,
                tc.tile_pool(name="a1_out", bufs=2) as pout,
                tc.tile_pool(name="a1_ps", bufs=2, space="PSUM") as pps,
            ):
                spv = s_pad[v & 1]
                for y0 in range(0, H, 32):
                    pk = load_pack(pin, spv, y0, 0, PW2, "s5a")
                    xh_st = pout.tile([C, 32, W], BF16, tag="xh_st", name="xh_st")
                    for blk in range(4):
                        ps = pps.tile([C, 4, 512], F32, tag="a1ps", name="a1ps")
                        for dy in range(KK):
                            for j in range(4):
                                rr = 8 * blk + 2 * j + dy
                                nc.tensor.matmul(
                                    ps[:, j, :], hT5v[it][:, dy, :],
                                    pk[:, rr:rr + 2, 2:258],
                                    start=(dy == 0), stop=(dy == KK - 1),
                                )
                        dst = xh_st[:, 8 * blk:8 * blk + 8, :].rearrange(
                            "c (j two) x -> c j (two x)", j=4)
                        if blk % 2 == 0:
                            nc.vector.tensor_copy(dst, ps[:])
                        else:
                            nc.scalar.copy(dst, ps[:])
                    eng = nc.scalar if (y0 // 32) % 2 == 0 else nc.sync
                    eng.dma_start(
                        xh_pad[:, 2 + y0:2 + y0 + 32, 2:258], xh_st[:]
                    )

        def phase_a2(it, v):
            """num/den + s update: sT[v+1] = sT[v] * num / den; s_pad[(v+1)&1]."""
            with (
                tc.tile_pool(name="a2_in", bufs=3) as pin,
                tc.tile_pool(name="a2_sb", bufs=2) as psb,
                tc.tile_pool(name="a2_ps", bufs=2, space="PSUM") as pps,
            ):
                pending = []
                x2v = x2d[:].rearrange("p (r c2) -> p r c2", r=PW2, c2=PW2)
                xhv = xh_pad
                for y0 in range(0, H, 32):
                    x2blk = pin.tile([128, 36, PW2], BF16, tag="x2b", name="x2b")
                    nc.sync.dma_start(x2blk[:], x2v[:, y0:y0 + 36, :])
                    xh2blk = pin.tile([128, 36, PW2], BF16, tag="xh2b", name="xh2b")
                    nc.scalar.dma_start(xh2blk[0:64, :, :], xhv[:, y0:y0 + 36, :])
                    nc.gpsimd.dma_start(
                        xh2blk[64:128, :, 0:PW2 - 1], xhv[:, y0:y0 + 36, 1:PW2]
                    )
                    for cb in range(2):
                        x0 = cb * 128
                        pnum = pps.tile([128, 32, S], F32, tag="pnum", name="pnum")
                        pden = pps.tile([128, 32, S], F32, tag="pden", name="pden")
                        for ps, blk in ((pnum, x2blk), (pden, xh2blk)):
                            nc.tensor.matmul(ps[:], zl[0:1, :], zt[0:1, 0:512],
                                             start=True, stop=False)
                            for ri in range(36):
                                y = y0 - 2 + ri
                                alo = max(0, y0 - (y - 2))
                                ahi = min(5, y0 + 32 - (y - 2))
                                z0 = (y - 2 - y0) + alo
                                for vi, va in enumerate((0, 2, 4)):
                                    kp = 128 if va < 4 else 64
                                    nc.tensor.matmul(
                                        ps[:, z0:z0 + ahi - alo, :].rearrange(
                                            "p z s -> p (z s)"),
                                        blk[0:kp, ri, x0 + va:x0 + va + 128],
                                        wnum[it][vi][0:kp, alo:ahi,
                                                     :].rearrange(
                                            "k a s -> k (a s)"),
                                        start=False,
                                        stop=(ri == 35 and va == 4),
                                    )
                        rec = psb.tile([128, 32, S], F32, tag="rec", name="rec")
                        nc.vector.reciprocal(rec[:], pden[:])
                        rat = psb.tile([128, 32, S], F32, tag="rat", name="rat")
                        nc.vector.tensor_mul(rat[:], pnum[:], rec[:])
                        nc.vector.tensor_mul(
                            sT[v + 1][cb][:, y0:y0 + 32, :],
                            sT[v][cb][:, y0:y0 + 32, :],
                            rat[:],
                        )
                        pending.append((cb, y0))
                        if len(pending) > 1:
                            pcb, py0 = pending.pop(0)
                            sT_to_dram(v + 1, pcb, py0, psb,
                                       nc.scalar if pcb == 0 else nc.vector,
                                       nc.sync if pcb == 0 else nc.scalar)
                for pcb, py0 in pending:
                    sT_to_dram(v + 1, pcb, py0, psb,
                               nc.scalar if pcb == 0 else nc.vector,
                               nc.sync if pcb == 0 else nc.scalar)

        def phase_b(it, v1):
            """xh_b^T = conv(s_pad[v1], h) into xxT; sconv -> h update."""
            with (
                tc.tile_pool(name="b_xx", bufs=1) as pxx,
                tc.tile_pool(name="b_in", bufs=2) as pin,
                tc.tile_pool(name="b_sr", bufs=1) as psr,
                tc.tile_pool(name="b_ps", bufs=1, space="PSUM") as pps,
                tc.tile_pool(name="b_srp", bufs=2, space="PSUM") as psrp,
                tc.tile_pool(name="b_acc", bufs=1, space="PSUM") as pacc,
                tc.tile_pool(name="b_fin", bufs=1) as pfin,
            ):
                spv = s_pad[v1 & 1]
                acc = pacc.tile([128, KK, KK, S], F32, tag="acc", name="acc")
                nmm = [0]
                total_acc = 2 * H
                for cb in range(2):
                    x0 = cb * 128
                    xxT = pxx.tile([128, H, 128], BF16, tag=f"xxT{cb}", name=f"xxT{cb}")
                    nc.sync.dma_start(xxT[:, 0:128, 0:64], xTd[cb, :, 0:128, :])
                    nc.scalar.dma_start(xxT[:, 128:256, 0:64], xTd[cb, :, 128:256, :])
                    for y0 in range(0, H, 32):
                        pk = load_pack(pin, spv, y0, x0, 136, "s9b")
                        for sub in range(2):
                            yb = y0 + 16 * sub
                            psh = [
                                pps.tile([128, 16, 32], F32, tag=f"bps{h}",
                                         name=f"bps{h}")
                                for h in range(2)
                            ]
                            for ps in psh:
                                nc.tensor.matmul(ps[:], zl[0:1, :],
                                                 zt[0:1, 0:512],
                                                 start=True, stop=False)
                            for ri in range(20):
                                y = yb - 2 + ri
                                rr = 16 * sub + ri
                                alo = max(0, yb - (y - 2))
                                ahi = min(5, yb + 16 - (y - 2))
                                z0 = (y - 2 - yb) + alo
                                for h, ps in enumerate(psh):
                                    nc.tensor.matmul(
                                        ps[:, z0:z0 + ahi - alo, :].rearrange(
                                            "p z c -> p (z c)"),
                                        pk[0:80, rr, 2:130],
                                        wxh[it][h][0:80, alo:ahi, :].rearrange(
                                            "k a c -> k (a c)"),
                                        start=False, stop=(ri == 19),
                                    )
                            for h, ps in enumerate(psh):
                                if (sub + h) % 2 == 0:
                                    nc.vector.tensor_copy(
                                        xxT[:, yb:yb + 16,
                                            64 + 32 * h:96 + 32 * h],
                                        ps[:])
                                else:
                                    nc.scalar.copy(
                                        xxT[:, yb:yb + 16,
                                            64 + 32 * h:96 + 32 * h],
                                        ps[:])
                        # srow: transpose s' rows for this block
                        srow = psr.tile([128, 36, 80], BF16, tag="srow", name="srow")
                        for bq in range(9):
                            sp_ps = psrp.tile([128, 4, 80], BF16, tag="srp", name="srp")
                            for j in range(4):
                                nc.tensor.transpose(
                                    sp_ps[:, j, :],
                                    pk[0:80, 4 * bq + j, 2:130],
                                    identt[0:80, 0:80],
                                )
                            if (bq + y0 // 32) % 2 == 0:
                                nc.scalar.copy(srow[:, 4 * bq:4 * bq + 4, :], sp_ps[:])
                            else:
                                nc.vector.tensor_copy(srow[:, 4 * bq:4 * bq + 4, :], sp_ps[:])
                        for z in range(32):
                            nc.tensor.matmul(
                                acc[:].rearrange("p a g s -> p (a g s)"),
                                xxT[:, y0 + z, :],
                                srow[:, z:z + 5, :].rearrange(
                                    "p r g -> p (r g)"),
                                start=(nmm[0] == 0),
                                stop=(nmm[0] == total_acc - 1),
                                skip_group_check=True,
                            )
                            nmm[0] += 1
                # ---- h update ----
                a_t = pfin.tile([C, 400], F32, tag="a_t", name="a_t")
                nc.scalar.copy(a_t[:], acc[0:C, :, :, :])
                bhi = pfin.tile([128, 400], F32, tag="bhi", name="bhi")
                nc.scalar.copy(bhi[C:128, :], acc[C:128, :, :, :])
                blo = pfin.tile([C, 400], F32, tag="blo", name="blo")
                nc.sync.dma_start(blo[:], bhi[C:128, :])
                rec = pfin.tile([C, 400], F32, tag="recb", name="recb")
                nc.vector.reciprocal(rec[:], blo[:])
                rr = pfin.tile([C, 400], F32, tag="rr", name="rr")
                nc.vector.tensor_mul(rr[:], a_t[:], rec[:])
                rrv = (
                    rr[:]
                    .rearrange("c (u v i) -> c u v i", u=KK, v=KK, i=S)
                    .rearrange("c u v i -> c i u v")
                )
                ho = hv[it][:].rearrange("c (i u v) -> c i u v", i=S, u=KK, v=KK)
                hn = hv[it + 1][:].rearrange("c (i u v) -> c i u v", i=S, u=KK, v=KK)
                nc.vector.tensor_mul(hn, ho, rrv)
                nc.vector.tensor_copy(hbv[it + 1][:], hv[it + 1][:])

        def s_init():
            with (
                tc.tile_pool(name="si_in", bufs=3) as pin,
                tc.tile_pool(name="si_sb", bufs=2) as psb,
                tc.tile_pool(name="si_ps", bufs=2, space="PSUM") as pps,
            ):
                x2v = x2d[:].rearrange("p (r c2) -> p r c2", r=PW2, c2=PW2)
                for y0 in range(0, H, 32):
                    x2i = pin.tile([C, 32, PW2], BF16, tag="x2i", name="x2i")
                    nc.sync.dma_start(x2i[:], x2v[0:C, y0 + 2:y0 + 34, :])
                    for cb in range(2):
                        x0 = cb * 128
                        ps = pps.tile([128, 32, S], F32, tag="ips", name="ips")
                        for z in range(32):
                            nc.tensor.matmul(
                                ps[:, z, :],
                                x2i[:, z, x0 + 2:x0 + 130],
                                wlt[:],
                                start=True, stop=True,
                            )
                        nc.vector.tensor_add(
                            sT[0][cb][:, y0:y0 + 32, :], ps[:], bpt[:]
                        )
                        sT_to_dram(0, cb, y0, psb,
                                   nc.scalar if cb == 0 else nc.vector,
                                   nc.sync if cb == 0 else nc.scalar)

        def final_conv(v):
            build_hT5(2)
            with (
                tc.tile_pool(name="f_in", bufs=2) as pin,
                tc.tile_pool(name="f_out", bufs=2) as pout,
                tc.tile_pool(name="f_ps", bufs=2, space="PSUM") as pps,
            ):
                spv = s_pad[v & 1]
                odv = out_d[:].rearrange("c (r c2) -> c r c2", r=H, c2=W)
                for y0 in range(0, H, 32):
                    pk = load_pack(pin, spv, y0, 0, PW2, "sb5")
                    o_st = pout.tile([C, 32, W], F32, tag="o_st", name="o_st")
                    for blk in range(4):
                        ps = pps.tile([C, 4, 512], F32, tag="fps", name="fps")
                        for dy in range(KK):
                            for j in range(4):
                                rr_ = 8 * blk + 2 * j + dy
                                nc.tensor.matmul(
                                    ps[:, j, :], hT5v[2][:, dy, :],
                                    pk[:, rr_:rr_ + 2, 2:258],
                                    start=(dy == 0), stop=(dy == KK - 1),
                                )
                        dst = o_st[:, 8 * blk:8 * blk + 8, :].rearrange(
                            "c (j two) x -> c j (two x)", j=4)
                        if blk % 2 == 0:
                            nc.scalar.copy(dst, ps[:])
                        else:
                            nc.vector.tensor_copy(dst, ps[:])
                    eng = nc.sync if (y0 // 32) % 2 == 0 else nc.scalar
                    eng.dma_start(odv[:, y0:y0 + 32, :], o_st[:])

        # ---- program ----
        s_init()
        for it in range(2):
            build_weights(it)
            phase_a1(it, it)        # uses s_pad[it&1], h_it -> xh_pad
            phase_a2(it, it)        # sT[it+1], s_pad[(it+1)&1]
            phase_b(it, it + 1)     # sconv on s', h update -> hv[it+1]
        final_conv(2)


def _build_nc():
    nc = bacc.Bacc("TRN2", target_bir_lowering=False)
    x2d = nc.dram_tensor("x2d", [128, PW2 * PW2], BF16, kind="ExternalInput")
    xTd = nc.dram_tensor("xTd", [2, 128, H, 64], BF16, kind="ExternalInput")
    wlb = nc.dram_tensor("wlb", [C, S], BF16, kind="ExternalInput")
    bpat = nc.dram_tensor("bpat", [128, S * 32], F32, kind="ExternalInput")
    h0f = nc.dram_tensor("h0f", [C, 400], F32, kind="ExternalInput")
    h0b = nc.dram_tensor("h0b", [C, 400], BF16, kind="ExternalInput")
    identb = nc.dram_tensor("identb", [128, 128], BF16, kind="ExternalInput")
    zerob = nc.dram_tensor("zerob", [C, 4 * PW4], BF16, kind="ExternalInput")
    out_d = nc.dram_tensor("out", [C, H * W], F32, kind="ExternalOutput")
    with tile.TileContext(nc) as tc:
        _emit(nc, tc, (x2d, xTd, wlb, bpat, h0f, h0b, identb, zerob), out_d)
    nc.compile()
    return nc


def _make_in_maps(inputs):
    import ml_dtypes

    bf = ml_dtypes.bfloat16
    x = np.ascontiguousarray(inputs["x"], dtype=np.float32)
    h0 = np.asarray(inputs["h0"], dtype=np.float32)
    W_lin = np.asarray(inputs["W_lin"], dtype=np.float32)
    b_lin = np.asarray(inputs["b_lin"], dtype=np.float32).reshape(S)

    wlb = np.ascontiguousarray(W_lin.T).astype(bf)
    bpat = np.zeros((128, S, 32), np.float32)
    bpat[:, :, :] = b_lin[None, :, None]
    bpat = bpat.reshape(128, S * 32)
    h0f = np.ascontiguousarray(h0.reshape(C, 400))
    h0b = h0f.astype(bf)
    identb = np.eye(128, dtype=np.float32).astype(bf)
    zerob = np.zeros((C, 4 * PW4), bf)

    in_maps = []
    for b in range(B):
        xb = x[b]
        xp = np.zeros((C, PW2, PW2), np.float32)
        xp[:, 2:2 + H, 2:2 + W] = xb
        x2 = np.zeros((128, PW2, PW2), np.float32)
        x2[0:64] = xp
        x2[64:128, :, 0:PW2 - 1] = xp[:, :, 1:PW2]
        x2d = x2.astype(bf).reshape(128, PW2 * PW2)
        xTd = np.zeros((2, 128, H, 64), np.float32)
        for cb in range(2):
            # xTd[cb, m, Y, c] = x[c, Y, cb*128+m]
            xTd[cb] = xb[:, :, cb * 128:(cb + 1) * 128].transpose(2, 1, 0)
        in_maps.append({
            "x2d": np.ascontiguousarray(x2d),
            "xTd": np.ascontiguousarray(xTd.astype(bf)),
            "wlb": wlb, "bpat": bpat, "h0f": h0f, "h0b": h0b,
            "identb": identb, "zerob": zerob,
        })
    return in_maps


def kernel(x, h0, W_lin, b_lin):
    from concourse.bass_utils import run_bass_kernel_spmd

    if "nc" not in _CACHE:
        _CACHE["nc"] = _build_nc()
    nc = _CACHE["nc"]

    in_maps = _make_in_maps({"x": x, "h0": h0, "W_lin": W_lin, "b_lin": b_lin})
    res = run_bass_kernel_spmd(nc, in_maps, core_ids=list(range(N_CORES)))
    _CACHE["last_result"] = res
    out = np.stack(
        [res.results[b]["out"].reshape(C, H, W) for b in range(B)], axis=0
    )
    return out


# revision 16
# speedup vs baseline: 1.1359x; 1.1359x over previous
"""Trainium2 Bass kernel for nn_Deconv (NMF deconvolution, B=8, C=64, SRC=16, 256x256, K=5).

Data-parallel over batch: each of 8 NeuronCores runs one sample. All
convolutions use a transposed-output formulation: each matmul takes one
input-image row as the stationary operand (lhsT, partition dim = packed
channel/x-shift), streams a small weight matrix as the moving operand, and
scatters into PSUM columns indexed (out_channel, out_row) for a 32-row block.

DMA strategy (this revision): the x-shift-packed row windows are built with
five full-row-segment DMAs spread across five engine queues (sync, scalar,
vector, gpsimd) instead of five column-sliced loads serialized on the sync
queue; transposed s'-writeback and conv outputs are batched into one DMA per
32-row block via multi-dim access patterns; PSUM zero-fill matmuls are
removed (the first accumulating matmul clears the bank via start=True).

Per iteration:
  A1: xh_a = conv(s, h) into a 32-row SBUF staging tile, one DMA per block
      to the padded DRAM image.
  A2: num^T = conv(x2, hT) and den^T = conv(xh2, hT) fused per 32-row block
      with the multiplicative s-update (DVE) and the s'-to-DRAM writeback.
  B:  xh_b^T = conv(s', h) evicted into an interleaved (Y, c128) tile that
      also holds x^T, then the sconv pixel-contraction accumulates
      num2/den2 for both images in one PSUM bank; h-update follows.
Final: conv(s, h) in the normal formulation straight into the output.

bf16 everywhere on the PE (fp32 PSUM accumulation); measured end-to-end
relative error ~6e-3 vs the fp32 reference.
"""

import sys

sys.path.insert(0, "/opt/trn_rl_repo")

import numpy as np

import concourse.bass as bass  # noqa: F401
import concourse.tile as tile
from concourse import bacc, mybir

F32 = mybir.dt.float32
BF16 = mybir.dt.bfloat16

B, C, S = 8, 64, 16
H = W = 256
KK = 5
PW2 = 260   # pad-2 canvas (x2, xh_pad)
PW4 = 264   # pad-4 canvas (s_pad)
N_CORES = 8

_CACHE = {}


def _emit(nc, tc, ins, outs):
    from contextlib import ExitStack

    x2d, xTd, wlb, bpat, h0f, h0b, identb, zerob = ins
    out_d = outs

    # round-robin DMA issue engines for the 5-shift pack loads
    dma_engines = [nc.sync, nc.scalar, nc.gpsimd]

    ctx = ExitStack()
    with ctx:
        dram = ctx.enter_context(tc.tile_pool(name="dram", bufs=1, space="DRAM"))
        consts = ctx.enter_context(tc.tile_pool(name="consts", bufs=1))

        s_pad = [
            dram.tile([S, PW4, PW4], BF16, tag=f"s_pad{i}", name=f"s_pad{i}")
            for i in range(2)
        ]
        xh_pad = dram.tile([C, PW2, PW2], BF16, tag="xh_pad", name="xh_pad")

        # ---- consts ----
        identt = consts.tile([128, 128], BF16, tag="identb", name="identb")
        nc.sync.dma_start(identt[:], identb[:])
        wlt = consts.tile([C, S], BF16, tag="wlb", name="wlb")
        nc.sync.dma_start(wlt[:], wlb[:])
        bpt = consts.tile([128, 32, S], F32, tag="bpat", name="bpat")
        nc.sync.dma_start(bpt[:], bpat[:])

        zt = consts.tile([1, 512], BF16, tag="zmm", name="zmm")
        nc.gpsimd.memset(zt[:], 0.0)
        zl = consts.tile([1, 128], BF16, tag="zl", name="zl")
        nc.gpsimd.memset(zl[:], 0.0)

        hv2 = [consts.tile([C, 400], F32, tag=f"hv{v}", name=f"hv{v}") for v in range(2)]
        hbv2 = [consts.tile([C, 400], BF16, tag=f"hb{v}", name=f"hb{v}") for v in range(2)]
        hv = {0: hv2[0], 1: hv2[1], 2: hv2[0]}
        hbv = {0: hbv2[0], 1: hbv2[1], 2: hbv2[0]}
        nc.sync.dma_start(hv[0][:], h0f[:])
        nc.sync.dma_start(hbv[0][:], h0b[:])

        # s^T chunks, ping-pong by version parity: cols (s, Y)
        sT2 = [
            [consts.tile([128, H, S], BF16, tag=f"sT{v}_{cb}", name=f"sT{v}_{cb}")
             for cb in range(2)]
            for v in range(2)
        ]
        sT = {0: sT2[0], 1: sT2[1], 2: sT2[0]}

        # per-iter weights
        wnum = [
            [consts.tile([128 if va < 4 else 64, KK, S], BF16,
                         tag=f"wnum{it}_{va}", name=f"wnum{it}_{va}")
             for va in (0, 2, 4)]
            for it in range(2)
        ]
        wxh = [
            [consts.tile([80, KK, 32], BF16, tag=f"wxh{it}_{h}",
                         name=f"wxh{it}_{h}") for h in range(2)]
            for it in range(2)
        ]
        hbQI = consts.tile([C, KK, KK, S], BF16, tag="hbQI", name="hbQI")
        hT5v = [consts.tile([80, KK, C], BF16, tag=f"hT5_{i}", name=f"hT5_{i}")
                for i in range(3)]
        hB = consts.tile([C, 400], BF16, tag="hB", name="hB")

        # ---- zero rims (once) ----
        rim_engines = [nc.sync, nc.scalar, nc.sync, nc.scalar]
        for si, sp in enumerate(s_pad):
            spv = sp
            rim_engines[0].dma_start(spv[:, 0:4, :], zerob[0:S, 0:4 * PW4])
            rim_engines[1].dma_start(spv[:, 260:264, :], zerob[0:S, 0:4 * PW4])
            rim_engines[2].dma_start(spv[:, 4:260, 0:4], zerob[0:S, 0:1024])
            rim_engines[3].dma_start(spv[:, 4:260, 260:264], zerob[0:S, 0:1024])
        xhv_ = xh_pad
        rim_engines[0].dma_start(xhv_[:, 0:2, :], zerob[0:C, 0:2 * PW2])
        rim_engines[1].dma_start(xhv_[:, 258:260, :], zerob[0:C, 0:2 * PW2])
        rim_engines[2].dma_start(xhv_[:, 2:258, 0:2], zerob[0:C, 0:512])
        rim_engines[3].dma_start(xhv_[:, 2:258, 258:260], zerob[0:C, 0:512])

        def build_weights(it):
            """wnum[it], wxh[it] from hbv[it]."""
            hb4 = hbv[it][:].rearrange("c (i u v) -> c i u v", i=S, u=KK, v=KK)
            for vi, va in enumerate((0, 2, 4)):
                kp = 128 if va < 4 else 64
                for d in range(kp // 64):
                    nc.vector.tensor_copy(
                        wnum[it][vi][d * 64:(d + 1) * 64, :, :],
                        hb4[:, :, :, 4 - va - d].rearrange("c i u -> c u i"),
                    )
            nc.gpsimd.tensor_copy(
                hbQI[:], hbv[it][:].rearrange("c (i u v) -> c u v i",
                                              i=S, u=KK, v=KK))
            with tc.tile_pool(name=f"wps{it}", bufs=1, space="PSUM") as wps:
                wx_ps = wps.tile([80, KK, C], BF16, tag="wx_ps", name="wx_ps")
                for p in range(KK):
                    nc.tensor.transpose(
                        wx_ps[:, 4 - p, :],
                        hbQI[:, p, :, :].rearrange("c v i -> c (v i)"),
                        identt[0:C, 0:C],
                    )
                nc.scalar.copy(wxh[it][0][:], wx_ps[:, :, 0:32])
                nc.scalar.copy(wxh[it][1][:], wx_ps[:, :, 32:64])
            build_hT5(it)

        def build_hT5(idx):
            hb4x = hbv[idx][:].rearrange("c (i u v) -> c i u v", i=S, u=KK, v=KK)
            hBo = hB[:].rearrange("c (u g i) -> c u g i", u=KK, g=KK, i=S)
            nc.gpsimd.tensor_copy(hBo, hb4x.rearrange("c i u g -> c u g i"))
            with tc.tile_pool(name=f"hw{idx}", bufs=1, space="PSUM") as fwp:
                for u in range(KK):
                    pw = fwp.tile([80, C], BF16, tag="pw", name="pw")
                    nc.tensor.transpose(pw[:], hB[:, u * 80:u * 80 + 80],
                                        identt[0:C, 0:C])
                    nc.scalar.copy(hT5v[idx][:, u, :], pw[:])

        def load_pack(pool, spv, y0, x0, wd, tag):
            """[80, 36, wd] tile: pack[16g+i, r, x] = spv[i, y0+2+r, x0+g+x].

            Five full-row-segment DMAs spread across five engine queues.
            """
            pk = pool.tile([80, 36, wd], BF16, tag=tag, name=tag)
            for g in range(KK):
                w = min(wd, PW4 - x0 - g)
                dma_engines[(y0 // 32 + g) % 3].dma_start(
                    pk[g * S:(g + 1) * S, :, 0:w],
                    spv[:, y0 + 2:y0 + 38, x0 + g:x0 + g + w],
                )
            return pk

        def sT_to_dram(v, cb, y0, pool, eng, deng):
            """Write sT[v][cb] rows [y0, y0+32) to s_pad[v & 1] via PE transpose."""
            x0 = cb * 128
            with tc.tile_pool(name="tbp", bufs=2, space="PSUM") as tbp:
                tb_ps = tbp.tile([128, 4, 128], BF16, tag="tb_ps", name="tb_ps")
                for j in range(4):
                    nc.tensor.transpose(
                        tb_ps[:, j, :],
                        sT[v][cb][:, y0 + 8 * j:y0 + 8 * j + 8, :].rearrange(
                            "p y s -> p (y s)"),
                        identt[:],
                    )
                tb_sb = pool.tile([128, 4, 128], BF16, tag="tb_sb", name="tb_sb")
                if eng is nc.vector:
                    nc.vector.tensor_copy(tb_sb[:], tb_ps[:])
                else:
                    nc.scalar.copy(tb_sb[:], tb_ps[:])
            # DMA APs are limited to 3 dims, so one DMA per 8-row group,
            # spread across the three DMA-capable queues.
            jeng = [dma_engines[(y0 // 32 + cb + j) % 3] for j in range(4)]
            for j in range(4):
                dst = s_pad[v & 1][:, 4 + y0 + 8 * j:4 + y0 + 8 * j + 8,
                                   4 + x0:4 + x0 + 128].rearrange(
                    "i yy m -> yy i m")
                jeng[j].dma_start(dst, tb_sb[:, j, :])

        def phase_a1(it, v):
            """xh_a = conv(s_pad[v], h) -> xh_pad, normal formulation."""
            with (
                tc.tile_pool(name="a1_in", bufs=2) as pin,
                tc.tile_pool(name="a1_out", bufs=2) as pout,
                tc.tile_pool(name="a1_ps", bufs=2, space="PSUM") as pps,
            ):
                spv = s_pad[v & 1]
                for y0 in range(0, H, 32):
                    pk = load_pack(pin, spv, y0, 0, PW2, "s5a")
                    xh_st = pout.tile([C, 32, W], BF16, tag="xh_st", name="xh_st")
                    for blk in range(4):
                        ps = pps.tile([C, 4, 512], F32, tag="a1ps", name="a1ps")
                        for dy in range(KK):
                            for j in range(4):
                                rr = 8 * blk + 2 * j + dy
                                nc.tensor.matmul(
                                    ps[:, j, :], hT5v[it][:, dy, :],
                                    pk[:, rr:rr + 2, 2:258],
                                    start=(dy == 0), stop=(dy == KK - 1),
                                )
                        dst = xh_st[:, 8 * blk:8 * blk + 8, :].rearrange(
                            "c (j two) x -> c j (two x)", j=4)
                        nc.vector.tensor_copy(dst, ps[:])
                    eng = nc.scalar if (y0 // 32) % 2 == 0 else nc.sync
                    eng.dma_start(
                        xh_pad[:, 2 + y0:2 + y0 + 32, 2:258], xh_st[:]
                    )

        def phase_a2(it, v):
            """num/den + s update: sT[v+1] = sT[v] * num / den; s_pad[(v+1)&1]."""
            with (
                tc.tile_pool(name="a2_in", bufs=3) as pin,
                tc.tile_pool(name="a2_sb", bufs=2) as psb,
                tc.tile_pool(name="a2_ps", bufs=2, space="PSUM") as pps,
            ):
                pending = []
                x2v = x2d[:].rearrange("p (r c2) -> p r c2", r=PW2, c2=PW2)
                xhv = xh_pad
                for y0 in range(0, H, 32):
                    x2blk = pin.tile([128, 36, PW2], BF16, tag="x2b", name="x2b")
                    nc.sync.dma_start(x2blk[:], x2v[:, y0:y0 + 36, :])
                    xh2blk = pin.tile([128, 36, PW2], BF16, tag="xh2b", name="xh2b")
                    nc.scalar.dma_start(xh2blk[0:64, :, :], xhv[:, y0:y0 + 36, :])
                    nc.gpsimd.dma_start(
                        xh2blk[64:128, :, 0:PW2 - 1], xhv[:, y0:y0 + 36, 1:PW2]
                    )
                    for cb in range(2):
                        x0 = cb * 128
                        pnum = pps.tile([128, 32, S], F32, tag="pnum", name="pnum")
                        pden = pps.tile([128, 32, S], F32, tag="pden", name="pden")
                        for ps, blk in ((pnum, x2blk), (pden, xh2blk)):
                            nc.tensor.matmul(ps[:], zl[0:1, :], zt[0:1, 0:512],
                                             start=True, stop=False)
                            for ri in range(36):
                                y = y0 - 2 + ri
                                alo = max(0, y0 - (y - 2))
                                ahi = min(5, y0 + 32 - (y - 2))
                                z0 = (y - 2 - y0) + alo
                                for vi, va in enumerate((0, 2, 4)):
                                    kp = 128 if va < 4 else 64
                                    nc.tensor.matmul(
                                        ps[:, z0:z0 + ahi - alo, :].rearrange(
                                            "p z s -> p (z s)"),
                                        blk[0:kp, ri, x0 + va:x0 + va + 128],
                                        wnum[it][vi][0:kp, alo:ahi,
                                                     :].rearrange(
                                            "k a s -> k (a s)"),
                                        start=False,
                                        stop=(ri == 35 and va == 4),
                                    )
                        rec = psb.tile([128, 32, S], F32, tag="rec", name="rec")
                        nc.vector.reciprocal(rec[:], pden[:])
                        rat = psb.tile([128, 32, S], F32, tag="rat", name="rat")
                        nc.vector.tensor_mul(rat[:], pnum[:], rec[:])
                        nc.vector.tensor_mul(
                            sT[v + 1][cb][:, y0:y0 + 32, :],
                            sT[v][cb][:, y0:y0 + 32, :],
                            rat[:],
                        )
                        pending.append((cb, y0))
                        if len(pending) > 1:
                            pcb, py0 = pending.pop(0)
                            sT_to_dram(v + 1, pcb, py0, psb,
                                       nc.vector,
                                       nc.sync if pcb == 0 else nc.scalar)
                for pcb, py0 in pending:
                    sT_to_dram(v + 1, pcb, py0, psb,
                               nc.vector,
                               nc.sync if pcb == 0 else nc.scalar)

        def phase_b(it, v1):
            """xh_b^T = conv(s_pad[v1], h) into xxT; sconv -> h update."""
            with (
                tc.tile_pool(name="b_xx", bufs=1) as pxx,
                tc.tile_pool(name="b_in", bufs=2) as pin,
                tc.tile_pool(name="b_sr", bufs=1) as psr,
                tc.tile_pool(name="b_ps", bufs=1, space="PSUM") as pps,
                tc.tile_pool(name="b_srp", bufs=2, space="PSUM") as psrp,
                tc.tile_pool(name="b_acc", bufs=1, space="PSUM") as pacc,
                tc.tile_pool(name="b_fin", bufs=1) as pfin,
            ):
                spv = s_pad[v1 & 1]
                acc = pacc.tile([128, KK, KK, S], F32, tag="acc", name="acc")
                nmm = [0]
                total_acc = 2 * H
                for cb in range(2):
                    x0 = cb * 128
                    xxT = pxx.tile([128, H, 128], BF16, tag=f"xxT{cb}", name=f"xxT{cb}")
                    nc.sync.dma_start(xxT[:, 0:128, 0:64], xTd[cb, :, 0:128, :])
                    nc.scalar.dma_start(xxT[:, 128:256, 0:64], xTd[cb, :, 128:256, :])
                    for y0 in range(0, H, 32):
                        pk = load_pack(pin, spv, y0, x0, 136, "s9b")
                        for sub in range(2):
                            yb = y0 + 16 * sub
                            psh = [
                                pps.tile([128, 16, 32], F32, tag=f"bps{h}",
                                         name=f"bps{h}")
                                for h in range(2)
                            ]
                            for ps in psh:
                                nc.tensor.matmul(ps[:], zl[0:1, :],
                                                 zt[0:1, 0:512],
                                                 start=True, stop=False)
                            for ri in range(20):
                                y = yb - 2 + ri
                                rr = 16 * sub + ri
                                alo = max(0, yb - (y - 2))
                                ahi = min(5, yb + 16 - (y - 2))
                                z0 = (y - 2 - yb) + alo
                                for h, ps in enumerate(psh):
                                    nc.tensor.matmul(
                                        ps[:, z0:z0 + ahi - alo, :].rearrange(
                                            "p z c -> p (z c)"),
                                        pk[0:80, rr, 2:130],
                                        wxh[it][h][0:80, alo:ahi, :].rearrange(
                                            "k a c -> k (a c)"),
                                        start=False, stop=(ri == 19),
                                    )
                            for h, ps in enumerate(psh):
                                nc.vector.tensor_copy(
                                    xxT[:, yb:yb + 16,
                                        64 + 32 * h:96 + 32 * h],
                                    ps[:])
                        # srow: transpose s' rows for this block
                        srow = psr.tile([128, 36, 80], BF16, tag="srow", name="srow")
                        for bq in range(9):
                            sp_ps = psrp.tile([128, 4, 80], BF16, tag="srp", name="srp")
                            for j in range(4):
                                nc.tensor.transpose(
                                    sp_ps[:, j, :],
                                    pk[0:80, 4 * bq + j, 2:130],
                                    identt[0:80, 0:80],
                                )
                            nc.vector.tensor_copy(
                                srow[:, 4 * bq:4 * bq + 4, :], sp_ps[:])
                        for z in range(32):
                            nc.tensor.matmul(
                                acc[:].rearrange("p a g s -> p (a g s)"),
                                xxT[:, y0 + z, :],
                                srow[:, z:z + 5, :].rearrange(
                                    "p r g -> p (r g)"),
                                start=(nmm[0] == 0),
                                stop=(nmm[0] == total_acc - 1),
                                skip_group_check=True,
                            )
                            nmm[0] += 1
                # ---- h update ----
                a_t = pfin.tile([C, 400], F32, tag="a_t", name="a_t")
                nc.scalar.copy(a_t[:], acc[0:C, :, :, :])
                bhi = pfin.tile([128, 400], F32, tag="bhi", name="bhi")
                nc.scalar.copy(bhi[C:128, :], acc[C:128, :, :, :])
                blo = pfin.tile([C, 400], F32, tag="blo", name="blo")
                nc.sync.dma_start(blo[:], bhi[C:128, :])
                rec = pfin.tile([C, 400], F32, tag="recb", name="recb")
                nc.vector.reciprocal(rec[:], blo[:])
                rr = pfin.tile([C, 400], F32, tag="rr", name="rr")
                nc.vector.tensor_mul(rr[:], a_t[:], rec[:])
                rrv = (
                    rr[:]
                    .rearrange("c (u v i) -> c u v i", u=KK, v=KK, i=S)
                    .rearrange("c u v i -> c i u v")
                )
                ho = hv[it][:].rearrange("c (i u v) -> c i u v", i=S, u=KK, v=KK)
                hn = hv[it + 1][:].rearrange("c (i u v) -> c i u v", i=S, u=KK, v=KK)
                nc.vector.tensor_mul(hn, ho, rrv)
                nc.vector.tensor_copy(hbv[it + 1][:], hv[it + 1][:])

        def s_init():
            with (
                tc.tile_pool(name="si_in", bufs=3) as pin,
                tc.tile_pool(name="si_sb", bufs=2) as psb,
                tc.tile_pool(name="si_ps", bufs=2, space="PSUM") as pps,
            ):
                x2v = x2d[:].rearrange("p (r c2) -> p r c2", r=PW2, c2=PW2)
                for y0 in range(0, H, 32):
                    x2i = pin.tile([C, 32, PW2], BF16, tag="x2i", name="x2i")
                    nc.sync.dma_start(x2i[:], x2v[0:C, y0 + 2:y0 + 34, :])
                    for cb in range(2):
                        x0 = cb * 128
                        ps = pps.tile([128, 32, S], F32, tag="ips", name="ips")
                        for z in range(32):
                            nc.tensor.matmul(
                                ps[:, z, :],
                                x2i[:, z, x0 + 2:x0 + 130],
                                wlt[:],
                                start=True, stop=True,
                            )
                        nc.vector.tensor_add(
                            sT[0][cb][:, y0:y0 + 32, :], ps[:], bpt[:]
                        )
                        sT_to_dram(0, cb, y0, psb,
                                   nc.vector,
                                   nc.sync if cb == 0 else nc.scalar)

        def final_conv(v):
            build_hT5(2)
            with (
                tc.tile_pool(name="f_in", bufs=2) as pin,
                tc.tile_pool(name="f_out", bufs=2) as pout,
                tc.tile_pool(name="f_ps", bufs=2, space="PSUM") as pps,
            ):
                spv = s_pad[v & 1]
                odv = out_d[:].rearrange("c (r c2) -> c r c2", r=H, c2=W)
                for y0 in range(0, H, 32):
                    pk = load_pack(pin, spv, y0, 0, PW2, "sb5")
                    o_st = pout.tile([C, 32, W], F32, tag="o_st", name="o_st")
                    for blk in range(4):
                        ps = pps.tile([C, 4, 512], F32, tag="fps", name="fps")
                        for dy in range(KK):
                            for j in range(4):
                                rr_ = 8 * blk + 2 * j + dy
                                nc.tensor.matmul(
                                    ps[:, j, :], hT5v[2][:, dy, :],
                                    pk[:, rr_:rr_ + 2, 2:258],
                                    start=(dy == 0), stop=(dy == KK - 1),
                                )
                        dst = o_st[:, 8 * blk:8 * blk + 8, :].rearrange(
                            "c (j two) x -> c j (two x)", j=4)
                        nc.vector.tensor_copy(dst, ps[:])
                    eng = nc.sync if (y0 // 32) % 2 == 0 else nc.scalar
                    eng.dma_start(odv[:, y0:y0 + 32, :], o_st[:])

        # ---- program ----
        s_init()
        for it in range(2):
            build_weights(it)
            phase_a1(it, it)        # uses s_pad[it&1], h_it -> xh_pad
            phase_a2(it, it)        # sT[it+1], s_pad[(it+1)&1]
            phase_b(it, it + 1)     # sconv on s', h update -> hv[it+1]
        final_conv(2)


def _build_nc():
    nc = bacc.Bacc("TRN2", target_bir_lowering=False)
    x2d = nc.dram_tensor("x2d", [128, PW2 * PW2], BF16, kind="ExternalInput")
    xTd = nc.dram_tensor("xTd", [2, 128, H, 64], BF16, kind="ExternalInput")
    wlb = nc.dram_tensor("wlb", [C, S], BF16, kind="ExternalInput")
    bpat = nc.dram_tensor("bpat", [128, S * 32], F32, kind="ExternalInput")
    h0f = nc.dram_tensor("h0f", [C, 400], F32, kind="ExternalInput")
    h0b = nc.dram_tensor("h0b", [C, 400], BF16, kind="ExternalInput")
    identb = nc.dram_tensor("identb", [128, 128], BF16, kind="ExternalInput")
    zerob = nc.dram_tensor("zerob", [C, 4 * PW4], BF16, kind="ExternalInput")
    out_d = nc.dram_tensor("out", [C, H * W], F32, kind="ExternalOutput")
    with tile.TileContext(nc) as tc:
        _emit(nc, tc, (x2d, xTd, wlb, bpat, h0f, h0b, identb, zerob), out_d)
    nc.compile()
    return nc


def _make_in_maps(inputs):
    import ml_dtypes

    bf = ml_dtypes.bfloat16
    x = np.ascontiguousarray(inputs["x"], dtype=np.float32)
    h0 = np.asarray(inputs["h0"], dtype=np.float32)
    W_lin = np.asarray(inputs["W_lin"], dtype=np.float32)
    b_lin = np.asarray(inputs["b_lin"], dtype=np.float32).reshape(S)

    wlb = np.ascontiguousarray(W_lin.T).astype(bf)
    bpat = np.zeros((128, S, 32), np.float32)
    bpat[:, :, :] = b_lin[None, :, None]
    bpat = bpat.reshape(128, S * 32)
    h0f = np.ascontiguousarray(h0.reshape(C, 400))
    h0b = h0f.astype(bf)
    identb = np.eye(128, dtype=np.float32).astype(bf)
    zerob = np.zeros((C, 4 * PW4), bf)

    in_maps = []
    for b in range(B):
        xb = x[b]
        xp = np.zeros((C, PW2, PW2), np.float32)
        xp[:, 2:2 + H, 2:2 + W] = xb
        x2 = np.zeros((128, PW2, PW2), np.float32)
        x2[0:64] = xp
        x2[64:128, :, 0:PW2 - 1] = xp[:, :, 1:PW2]
        x2d = x2.astype(bf).reshape(128, PW2 * PW2)
        xTd = np.zeros((2, 128, H, 64), np.float32)
        for cb in range(2):
            # xTd[cb, m, Y, c] = x[c, Y, cb*128+m]
            xTd[cb] = xb[:, :, cb * 128:(cb + 1) * 128].transpose(2, 1, 0)
        in_maps.append({
            "x2d": np.ascontiguousarray(x2d),
            "xTd": np.ascontiguousarray(xTd.astype(bf)),
            "wlb": wlb, "bpat": bpat, "h0f": h0f, "h0b": h0b,
            "identb": identb, "zerob": zerob,
        })
    return in_maps


def kernel(x, h0, W_lin, b_lin):
    from concourse.bass_utils import run_bass_kernel_spmd

    if "nc" not in _CACHE:
        _CACHE["nc"] = _build_nc()
    nc = _CACHE["nc"]

    in_maps = _make_in_maps({"x": x, "h0": h0, "W_lin": W_lin, "b_lin": b_lin})
    res = run_bass_kernel_spmd(nc, in_maps, core_ids=list(range(N_CORES)))
    _CACHE["last_result"] = res
    out = np.stack(
        [res.results[b]["out"].reshape(C, H, W) for b in range(B)], axis=0
    )
    return out
